# revision 21
# baseline (speedup 1.0000x reference)
"""3-layer GCN on 8 TRN2 NeuronCores — multi-queue quad-gather + one-hot
matmul aggregation (v5: band-pure chunks).

- Each core owns 12500 dst nodes in 98 blocks of 128. Edges are grouped by
  (dst block, band=src%4) and each segment is padded to a multiple of 128
  ("chunks"), with the chunk structure shared across cores (max over cores).
- Feature tables are bf16 quad rows ([n/4, 128]: 4 node rows per 256B line).
  One dma_gather call per dst block (one 256B element per edge, landing the
  src quad at partition e%128); calls round-robin over 4 SWDGE queues so Q7
  descriptor generation runs on all four core pairs concurrently (~3x).
  Trailing pad slots of each call carry index -1 (ucode skips them).
- One-hot S[e, dst%128] for a whole call is built by a single DVE
  tensor_tensor is_equal: in0 = tiled iota 0..127 (fp16), in1 = per-chunk ids
  broadcast across 128 columns via a stride-0 AP; out bf16.
- Aggregation: per chunk one matmul, lhsT = band slice of the gathered quads
  [128e, 32f] bf16, rhs = S [128e, 128d], accumulating PSUM [32f, 128d] per
  block. Chunks are band-pure, so the lhsT slice selects each edge's row.
- Dense stage per block: 1 ACT copy PSUM->SBUF bf16, 1 matmul (k=32) against
  W, a k=1 ones-row matmul adds the bias in PSUM, LeakyReLU via 2 DVE ops.
- AllGather (bf16) rebuilds the full node table between layers.
"""

import json
import os

import numpy as np

import concourse.bacc as bacc
import concourse.bass as bass
import concourse.mybir as mybir
import concourse.tile as tile

N = 100000
NC = 8
OWN = 12500  # dst nodes per core
NBLK = 98  # ceil(OWN / 128)
SP = NBLK * 128  # 12544 padded node slots per core
F = 32
FO_L = [32, 32, 16]
NQ = 4  # SWDGE queues
XQ = N // 4  # x quad rows
ZQ = (NC * SP) // 4  # z quad rows
PAD_ID = 300.0  # one-hot id for padding slots (no match in [0, 128))
KBMAX = 24  # compile-time bound on chunks per block
GRP = 2  # blocks per gather call
KGMAX = 40  # bound on chunks per gather call

_cache = {}


# ---------------------------------------------------------------- BIR patch
def _split_sync_waits(bir_json, max_waits=1):
    d = json.loads(bir_json.decode() if isinstance(bir_json, (bytes, bytearray)) else bir_json)
    ctr = 0
    for f in d.get("functions", []):
        for bb in f.get("blocks", []):
            insts = bb.get("instructions", [])
            if not any(
                len((i.get("sync_info") or {}).get("on_wait") or []) > max_waits
                for i in insts
            ):
                continue
            out = []
            for inst in insts:
                si = inst.get("sync_info")
                waits = (si or {}).get("on_wait") or []
                if len(waits) > max_waits:
                    extra = waits[: len(waits) - max_waits]
                    si["on_wait"] = waits[len(waits) - max_waits :]
                    for w in extra:
                        ctr += 1
                        out.append(
                            {
                                "debug": inst.get("debug", 0),
                                "engine": inst["engine"],
                                "ins": [],
                                "outs": [],
                                "name": f"waitsplit-{ctr}",
                                "opcode": "NoOp",
                                "sync_info": {"on_update": [], "on_wait": [w]},
                            }
                        )
                out.append(inst)
            bb["instructions"] = out
    return json.dumps(d).encode()


_patched = False


def _install_birpatch():
    global _patched
    if _patched:
        return
    _patched = True
    import concourse.bass_utils as bu

    orig = bu.compile_bir_kernel

    def patched(bir_json, tmpdir, neff_name="file.neff"):
        return orig(_split_sync_waits(bir_json), tmpdir, neff_name=neff_name)

    bu.compile_bir_kernel = patched
    try:
        import concourse.bass2jax as b2j

        b2j.compile_bir_kernel = patched
    except ImportError:
        pass


# ------------------------------------------------------------- host planning
def _wrap_idx(arr):
    """[T] int16 -> [128, T/16], index i at [i%16 (replicated x8), i//16]."""
    T = arr.shape[0]
    w = arr.reshape(T // 16, 16).T
    return np.tile(w, (8, 1)).copy()


def _build_plan(src, dst):
    owner = dst // OWN
    nseg_all = np.zeros((NC, NBLK, 4), np.int64)
    core_edges = []
    for c in range(NC):
        m = owner == c
        es = src[m]
        ed = dst[m] - c * OWN
        blk = ed // 128
        band = es % 4
        order = np.argsort(blk * 4 + band, kind="stable")
        es, ed, blk, band = es[order], ed[order], blk[order], band[order]
        np.add.at(nseg_all[c], (blk, band), 1)
        core_edges.append((es, ed, blk, band))

    # shared chunk structure: per-(block, band) chunk count = max over cores
    kseg = -(-nseg_all.max(axis=0) // 128)  # [NBLK, 4]
    kb = kseg.sum(axis=1)  # chunks per block
    kseg[kb == 0, 0] = 1
    kb = kseg.sum(axis=1)
    assert kb.max() <= KBMAX, kb.max()
    cs = np.zeros(NBLK + 1, np.int64)
    cs[1:] = np.cumsum(kb)
    nch = int(cs[-1])
    T = nch * 128
    # chunk start of each (block, band) segment
    seg_cs = np.zeros((NBLK, 4), np.int64)
    seg_cs[:, 0] = cs[:-1]
    seg_cs[:, 1:] = cs[:-1, None] + np.cumsum(kseg, axis=1)[:, :-1]
    # per-chunk band
    ch_band = np.zeros(nch, np.int64)
    for b in range(NBLK):
        for j in range(4):
            ch_band[seg_cs[b, j] : seg_cs[b, j] + kseg[b, j]] = j

    per_core = []
    for c in range(NC):
        es, ed, blk, band = core_edges[c]
        nseg = nseg_all[c]
        first = np.zeros(NBLK * 4, np.int64)
        first[1:] = np.cumsum(nseg.reshape(-1))[:-1]
        within = np.arange(len(es)) - first[blk * 4 + band]
        pos = seg_cs[blk, band] * 128 + within

        gz = np.zeros(T, np.int16)
        ids = np.full(T, PAD_ID, np.float32)
        zrow = (es // OWN) * SP + (es % OWN)
        gz[pos] = (zrow // 4).astype(np.int16)
        ids[pos] = (ed % 128).astype(np.float32)
        per_core.append(
            {
                "gz": _wrap_idx(gz),
                "ids": ids.reshape(nch, 128).T.copy(),  # [128, nch] f32; cast in driver
            }
        )
    return {"kb": kb, "cs": cs, "nch": nch, "T": T, "ch_band": ch_band}, per_core


# --------------------------------------------------------------- bass build
def _build_nc(plan):
    nc = bacc.Bacc(
        "TRN2",
        target_bir_lowering=False,
        debug=False,
        num_devices=NC,
        num_swdge_queues=NQ,
    )
    f32 = mybir.dt.float32
    f16 = mybir.dt.float16
    bf16 = mybir.dt.bfloat16
    i16 = mybir.dt.int16
    kb, cs, nch = plan["kb"], plan["cs"], plan["nch"]
    ch_band = plan["ch_band"]
    T = plan["T"]

    xz = nc.dram_tensor("xz", [ZQ, 128], bf16, kind="ExternalInput")
    w_in = [
        nc.dram_tensor(f"w{i}", [F, FO_L[i]], bf16, kind="ExternalInput")
        for i in range(3)
    ]
    b_in = [
        nc.dram_tensor(f"b{i}", [1, FO_L[i]], bf16, kind="ExternalInput")
        for i in range(3)
    ]
    gidx_in = nc.dram_tensor("gz", [128, T // 16], i16, kind="ExternalInput")
    ids_in = nc.dram_tensor("ids", [128, nch], bf16, kind="ExternalInput")
    iota_in = nc.dram_tensor("iota", [128, KGMAX * 128], bf16, kind="ExternalInput")
    ones_in = nc.dram_tensor("ones", [1, 128], bf16, kind="ExternalInput")
    out = nc.dram_tensor("out", [SP, FO_L[2]], f32, kind="ExternalOutput")

    cc_in = [
        nc.dram_tensor(f"cc_in{i}", [SP, F], bf16, kind="Internal") for i in range(2)
    ]
    cc_out = [
        nc.dram_tensor(f"cc_out{i}", [NC * SP, F], bf16, kind="Internal", addr_space="Shared")
        for i in range(2)
    ]
    cc_loc = [
        nc.dram_tensor(f"cc_loc{i}", [NC * SP, F], bf16, kind="Internal")
        for i in range(2)
    ]

    def src_ap(layer, early=False):
        if layer == 0:
            t = xz[:].tensor
        elif early:
            t = cc_out[layer - 1][:].tensor
        else:
            t = cc_loc[layer - 1][:].tensor
        return bass.AP(t, 0, [[128, ZQ], [1, 128]])

    with tile.TileContext(nc) as tc:
        with (
            tc.tile_pool(name="consts", bufs=1) as constp,
            tc.tile_pool(name="idx", bufs=1) as idxp,
            tc.tile_pool(name="gat", bufs=10) as gatp,
            tc.tile_pool(name="onehot", bufs=3) as sp_,
            tc.tile_pool(name="aggt", bufs=4) as aggp,
            tc.tile_pool(name="zz", bufs=6) as zzp,
            tc.tile_pool(name="stg", bufs=2) as stgp,
            tc.tile_pool(name="psA", bufs=4, space="PSUM") as psA,
            tc.tile_pool(name="psZ", bufs=3, space="PSUM") as psZ,
        ):
            w_t = []
            b_t = []
            for i in range(3):
                wt = constp.tile([F, FO_L[i]], bf16, tag=f"w{i}")
                bt = constp.tile([1, FO_L[i]], bf16, tag=f"b{i}")
                nc.sync.dma_start(wt[:], w_in[i][:])
                nc.sync.dma_start(bt[:], b_in[i][:])
                w_t.append(wt)
                b_t.append(bt)
            iota_t = constp.tile([128, KGMAX * 128], bf16, tag="iota")
            nc.sync.dma_start(iota_t[:], iota_in[:])
            ones_t = constp.tile([1, 128], bf16, tag="ones")
            nc.sync.dma_start(ones_t[:], ones_in[:])

            gidx_t = idxp.tile([128, T // 16], i16, tag="gidx", name="gidx_t")
            nc.sync.dma_start(gidx_t[:], gidx_in[:])
            ids_t = idxp.tile([128, nch], bf16, tag="ids")
            nc.sync.dma_start(ids_t[:], ids_in[:])

            def dense(layer, b, psum):
                fo = FO_L[layer]
                aggT = aggp.tile([32, 128], bf16, tag="aggT")
                nc.scalar.copy(aggT[:], psum[:])
                zp = psZ.tile([128, fo], f32, tag="z")
                nc.tensor.matmul(
                    zp[:], lhsT=aggT[:], rhs=w_t[layer][:], start=True, stop=False
                )
                nc.tensor.matmul(
                    zp[:], lhsT=ones_t[:], rhs=b_t[layer][:], start=False, stop=True
                )
                r0, r1 = b * 128, (b + 1) * 128
                if layer < 2:
                    zm = zzp.tile([128, fo], f32, tag="zm")
                    nc.vector.tensor_scalar_mul(zm[:], zp[:], 0.1)
                    zz = zzp.tile([128, fo], bf16, tag="zz16")
                    nc.vector.tensor_tensor(
                        out=zz[:], in0=zp[:], in1=zm[:], op=mybir.AluOpType.max
                    )
                    nc.sync.dma_start(cc_in[layer][r0:r1, :], zz[:])
                else:
                    zz = zzp.tile([128, fo], f32, tag="zz32")
                    nc.scalar.copy(zz[:], zp[:])
                    nc.sync.dma_start(out[r0:r1, :], zz[:])

            for layer in range(3):
                fo = FO_L[layer]
                for p0 in range(0, NBLK, GRP):
                    blocks = list(range(p0, min(p0 + GRP, NBLK)))
                    a = int(cs[blocks[0]])  # first chunk of group
                    k = int(cs[blocks[-1] + 1] - cs[blocks[0]])  # chunks in group
                    g = gatp.tile([128, KGMAX * 128], bf16, tag="g")
                    g3 = g[:, : k * 128].rearrange("p (c f) -> p c f", f=128)
                    nc.gpsimd.dma_gather(
                        out_ap=g3,
                        in_ap=src_ap(layer),
                        idxs_ap=gidx_t[:, a * 8 : (a + k) * 8],
                        num_idxs=k * 128,
                        num_idxs_reg=k * 128,
                        elem_size=128,
                        single_packet=False,
                        queue_num=(p0 // GRP) % NQ,
                    )
                    # batched dst one-hot for all k chunks of this group
                    sb = sp_.tile([128, KGMAX * 128], bf16, tag="s")
                    s3 = sb[:, : k * 128].rearrange("p (c n) -> p c n", n=128)
                    idsl = ids_t[:, a : a + k]
                    in1 = bass.AP(
                        idsl.tensor, idsl.offset, [idsl.ap[0], idsl.ap[1], [0, 128]]
                    )
                    in0 = iota_t[:, : k * 128].rearrange("p (c n) -> p c n", n=128)
                    nc.vector.tensor_tensor(
                        out=s3, in0=in0, in1=in1, op=mybir.AluOpType.is_equal
                    )
                    for b in blocks:
                        kbb = int(kb[b])
                        ab = int(cs[b]) - a  # chunk offset within group
                        psum = psA.tile([32, 128], f32, tag="agg")
                        for ci in range(kbb):
                            cc = ab + ci
                            j = int(ch_band[a + cc])
                            nc.tensor.matmul(
                                psum[:],
                                lhsT=g3[:, cc, 32 * j : 32 * j + 32],
                                rhs=s3[:, cc, :],
                                start=(ci == 0),
                                stop=(ci == kbb - 1),
                            )
                        dense(layer, b, psum)

                if layer < 2:
                    nc.gpsimd.collective_compute(
                        "AllGather",
                        mybir.AluOpType.bypass,
                        ins=[cc_in[layer][:]],
                        outs=[cc_out[layer][:]],
                        replica_groups=[list(range(NC))],
                    )
                    # gathers from the Shared window are ~60% slower than
                    # from plain local DRAM; stage a local copy through SBUF
                    # (DRAM->DRAM DMA corrupts; SBUF staging is tracked+safe).
                    CW = (NC * SP * F) // 128  # 25088 cols as [128, CW] bf16
                    for r in range(16):
                        stg = stgp.tile([128, CW // 16], bf16, tag="stg")
                        src = bass.AP(
                            cc_out[layer][:].tensor, r * (CW // 16),
                            [[CW, 128], [1, CW // 16]],
                        )
                        dst = bass.AP(
                            cc_loc[layer][:].tensor, r * (CW // 16),
                            [[CW, 128], [1, CW // 16]],
                        )
                        nc.sync.dma_start(stg[:], src)
                        nc.sync.dma_start(dst, stg[:])
    nc.compile()
    return nc


# ------------------------------------------------------------------- driver
def kernel(**inputs):
    _install_birpatch()
    import ml_dtypes

    bf = ml_dtypes.bfloat16
    x = np.asarray(inputs["x"], np.float32)
    src = np.asarray(inputs["src"], np.int64)
    dst = np.asarray(inputs["dst"], np.int64)
    Ws = [np.asarray(inputs[k], np.float32) for k in ("W1", "W2", "W3")]
    bs = [np.asarray(inputs[k], np.float32) for k in ("b1", "b2", "b3")]

    key = hash((src.tobytes(), dst.tobytes()))
    if key not in _cache:
        plan, per_core = _build_plan(src, dst)
        nc = _build_nc(plan)
        _cache[key] = (nc, plan, per_core)
    nc, plan, per_core = _cache[key]

    xz = np.zeros((NC * SP, F), np.float32)
    for c in range(NC):
        xz[c * SP : c * SP + OWN] = x[c * OWN : (c + 1) * OWN]
    xzv = xz.astype(bf).reshape(ZQ, 128)
    iota = np.tile(np.arange(128), (128, KGMAX)).astype(bf)
    ones = np.ones((1, 128), bf)

    in_maps = []
    for c in range(NC):
        pc = per_core[c]
        m = {
            "xz": xzv,
            "gz": pc["gz"],
            "ids": pc["ids"].astype(bf),
            "iota": iota,
            "ones": ones,
        }
        for i in range(3):
            m[f"w{i}"] = Ws[i].astype(bf)
            m[f"b{i}"] = bs[i].astype(bf)[None, :]
        in_maps.append(m)

    from concourse.bass_utils import run_bass_kernel_spmd

    trace = os.environ.get("GCN_TRACE") == "1"
    res = run_bass_kernel_spmd(nc, in_maps, core_ids=list(range(NC)), trace=trace)
    global last_exec_ns
    last_exec_ns = res.exec_time_ns

    out = np.empty((N, FO_L[2]), np.float32)
    for c in range(NC):
        out[c * OWN : (c + 1) * OWN] = res.results[c]["out"][:OWN]
    return out


# revision 24
# speedup vs baseline: 1.2039x; 1.2039x over previous
"""3-layer GCN on 8 TRN2 NeuronCores — multi-queue quad-gather + one-hot
matmul aggregation (v5: band-pure chunks).

- Each core owns 12500 dst nodes in 98 blocks of 128. Edges are grouped by
  (dst block, band=src%4) and each segment is padded to a multiple of 128
  ("chunks"), with the chunk structure shared across cores (max over cores).
- Feature tables are bf16 quad rows ([n/4, 128]: 4 node rows per 256B line).
  One dma_gather call per dst block (one 256B element per edge, landing the
  src quad at partition e%128); calls round-robin over 4 SWDGE queues so Q7
  descriptor generation runs on all four core pairs concurrently (~3x).
  Trailing pad slots of each call carry index -1 (ucode skips them).
- One-hot S[e, dst%128] for a whole call is built by a single DVE
  tensor_tensor is_equal: in0 = tiled iota 0..127 (fp16), in1 = per-chunk ids
  broadcast across 128 columns via a stride-0 AP; out bf16.
- Aggregation: per chunk one matmul, lhsT = band slice of the gathered quads
  [128e, 32f] bf16, rhs = S [128e, 128d], accumulating PSUM [32f, 128d] per
  block. Chunks are band-pure, so the lhsT slice selects each edge's row.
- Dense stage per block: 1 ACT copy PSUM->SBUF bf16, 1 matmul (k=32) against
  W, a k=1 ones-row matmul adds the bias in PSUM, LeakyReLU via 2 DVE ops.
- AllGather (bf16) rebuilds the full node table between layers.
"""

import json
import os

import numpy as np

import concourse.bacc as bacc
import concourse.bass as bass
import concourse.mybir as mybir
import concourse.tile as tile

N = 100000
NC = 8
OWN = 12500  # dst nodes per core
NBLK = 98  # ceil(OWN / 128)
SP = NBLK * 128  # 12544 padded node slots per core
F = 32
FO_L = [32, 32, 16]
NQ = 4  # SWDGE queues
XQ = N // 4  # x quad rows
ZQ = (NC * SP) // 4  # z quad rows
PAD_ID = 300.0  # one-hot id for padding slots (no match in [0, 128))
KBMAX = 24  # compile-time bound on chunks per block
GRP = 2  # blocks per gather call
KGMAX = 44  # bound on chunks per gather call

_cache = {}


# ---------------------------------------------------------------- BIR patch
def _split_sync_waits(bir_json, max_waits=1):
    d = json.loads(bir_json.decode() if isinstance(bir_json, (bytes, bytearray)) else bir_json)
    ctr = 0
    for f in d.get("functions", []):
        for bb in f.get("blocks", []):
            insts = bb.get("instructions", [])
            if not any(
                len((i.get("sync_info") or {}).get("on_wait") or []) > max_waits
                for i in insts
            ):
                continue
            out = []
            for inst in insts:
                si = inst.get("sync_info")
                waits = (si or {}).get("on_wait") or []
                if len(waits) > max_waits:
                    extra = waits[: len(waits) - max_waits]
                    si["on_wait"] = waits[len(waits) - max_waits :]
                    for w in extra:
                        ctr += 1
                        out.append(
                            {
                                "debug": inst.get("debug", 0),
                                "engine": inst["engine"],
                                "ins": [],
                                "outs": [],
                                "name": f"waitsplit-{ctr}",
                                "opcode": "NoOp",
                                "sync_info": {"on_update": [], "on_wait": [w]},
                            }
                        )
                out.append(inst)
            bb["instructions"] = out
    return json.dumps(d).encode()


_patched = False


def _install_birpatch():
    global _patched
    if _patched:
        return
    _patched = True
    import concourse.bass_utils as bu

    orig = bu.compile_bir_kernel

    def patched(bir_json, tmpdir, neff_name="file.neff"):
        return orig(_split_sync_waits(bir_json), tmpdir, neff_name=neff_name)

    bu.compile_bir_kernel = patched
    try:
        import concourse.bass2jax as b2j

        b2j.compile_bir_kernel = patched
    except ImportError:
        pass


# ------------------------------------------------------------- host planning
def _wrap_idx(arr):
    """[T] int16 -> [128, T/16], index i at [i%16 (replicated x8), i//16]."""
    T = arr.shape[0]
    w = arr.reshape(T // 16, 16).T
    return np.tile(w, (8, 1)).copy()


def _build_plan(src, dst):
    owner = dst // OWN
    nseg_all = np.zeros((NC, NBLK, 4), np.int64)
    core_edges = []
    for c in range(NC):
        m = owner == c
        es = src[m]
        ed = dst[m] - c * OWN
        blk = ed // 128
        band = es % 4
        order = np.argsort(blk * 4 + band, kind="stable")
        es, ed, blk, band = es[order], ed[order], blk[order], band[order]
        np.add.at(nseg_all[c], (blk, band), 1)
        core_edges.append((es, ed, blk, band))

    # shared chunk structure: per-(block, band) chunk count = max over cores
    kseg = -(-nseg_all.max(axis=0) // 128)  # [NBLK, 4]
    kb = kseg.sum(axis=1)  # chunks per block
    kseg[kb == 0, 0] = 1
    kb = kseg.sum(axis=1)
    assert kb.max() <= KBMAX, kb.max()
    cs = np.zeros(NBLK + 1, np.int64)
    cs[1:] = np.cumsum(kb)
    nch = int(cs[-1])
    T = nch * 128
    # chunk start of each (block, band) segment
    seg_cs = np.zeros((NBLK, 4), np.int64)
    seg_cs[:, 0] = cs[:-1]
    seg_cs[:, 1:] = cs[:-1, None] + np.cumsum(kseg, axis=1)[:, :-1]
    # per-chunk band
    ch_band = np.zeros(nch, np.int64)
    for b in range(NBLK):
        for j in range(4):
            ch_band[seg_cs[b, j] : seg_cs[b, j] + kseg[b, j]] = j

    per_core = []
    for c in range(NC):
        es, ed, blk, band = core_edges[c]
        nseg = nseg_all[c]
        first = np.zeros(NBLK * 4, np.int64)
        first[1:] = np.cumsum(nseg.reshape(-1))[:-1]
        within = np.arange(len(es)) - first[blk * 4 + band]
        pos = seg_cs[blk, band] * 128 + within

        gz = np.zeros(T, np.int16)
        ids = np.full(T, PAD_ID, np.float32)
        zrow = (es // OWN) * SP + (es % OWN)
        gz[pos] = (zrow // 4).astype(np.int16)
        ids[pos] = (ed % 128).astype(np.float32)
        per_core.append(
            {
                "gz": _wrap_idx(gz),
                "ids": ids.reshape(nch, 128).T.copy(),  # [128, nch] f32; cast in driver
            }
        )
    return {"kb": kb, "cs": cs, "nch": nch, "T": T, "ch_band": ch_band}, per_core


# --------------------------------------------------------------- bass build
def _build_nc(plan):
    nc = bacc.Bacc(
        "TRN2",
        target_bir_lowering=False,
        debug=False,
        num_devices=NC,
        num_swdge_queues=NQ,
    )
    f32 = mybir.dt.float32
    f16 = mybir.dt.float16
    bf16 = mybir.dt.bfloat16
    i16 = mybir.dt.int16
    kb, cs, nch = plan["kb"], plan["cs"], plan["nch"]
    ch_band = plan["ch_band"]
    T = plan["T"]

    xz = nc.dram_tensor("xz", [ZQ, 128], bf16, kind="ExternalInput")
    w_in = [
        nc.dram_tensor(f"w{i}", [F, FO_L[i]], bf16, kind="ExternalInput")
        for i in range(3)
    ]
    b_in = [
        nc.dram_tensor(f"b{i}", [1, FO_L[i]], bf16, kind="ExternalInput")
        for i in range(3)
    ]
    gidx_in = nc.dram_tensor("gz", [128, T // 16], i16, kind="ExternalInput")
    ids_in = nc.dram_tensor("ids", [128, nch], bf16, kind="ExternalInput")
    iota_in = nc.dram_tensor("iota", [128, KGMAX * 128], bf16, kind="ExternalInput")
    ones_in = nc.dram_tensor("ones", [1, 128], bf16, kind="ExternalInput")
    out = nc.dram_tensor("out", [SP, FO_L[2]], f32, kind="ExternalOutput")

    cc_in = [
        nc.dram_tensor(f"cc_in{i}", [SP, F], bf16, kind="Internal") for i in range(2)
    ]
    cc_out = [
        nc.dram_tensor(f"cc_out{i}", [NC * SP, F], bf16, kind="Internal", addr_space="Shared")
        for i in range(2)
    ]
    cc_loc = [
        nc.dram_tensor(f"cc_loc{i}", [NC * SP, F], bf16, kind="Internal")
        for i in range(2)
    ]

    def src_ap(layer, early=False):
        if layer == 0:
            t = xz[:].tensor
        elif early:
            t = cc_out[layer - 1][:].tensor
        else:
            t = cc_loc[layer - 1][:].tensor
        return bass.AP(t, 0, [[128, ZQ], [1, 128]])

    with tile.TileContext(nc) as tc:
        with (
            tc.tile_pool(name="consts", bufs=1) as constp,
            tc.tile_pool(name="idx", bufs=1) as idxp,
            tc.tile_pool(name="gat", bufs=8) as gatp,
            tc.tile_pool(name="onehot", bufs=4) as sp_,
            tc.tile_pool(name="aggt", bufs=4) as aggp,
            tc.tile_pool(name="zz", bufs=6) as zzp,
            tc.tile_pool(name="stg", bufs=2) as stgp,
            tc.tile_pool(name="psA", bufs=5, space="PSUM") as psA,
            tc.tile_pool(name="psZ", bufs=3, space="PSUM") as psZ,
        ):
            w_t = []
            b_t = []
            for i in range(3):
                wt = constp.tile([F, FO_L[i]], bf16, tag=f"w{i}")
                bt = constp.tile([1, FO_L[i]], bf16, tag=f"b{i}")
                nc.sync.dma_start(wt[:], w_in[i][:])
                nc.sync.dma_start(bt[:], b_in[i][:])
                w_t.append(wt)
                b_t.append(bt)
            iota_t = constp.tile([128, KGMAX * 128], bf16, tag="iota")
            nc.sync.dma_start(iota_t[:], iota_in[:])
            ones_t = constp.tile([1, 128], bf16, tag="ones")
            nc.sync.dma_start(ones_t[:], ones_in[:])

            gidx_t = idxp.tile([128, T // 16], i16, tag="gidx", name="gidx_t")
            nc.sync.dma_start(gidx_t[:], gidx_in[:])
            ids_t = idxp.tile([128, nch], bf16, tag="ids")
            nc.sync.dma_start(ids_t[:], ids_in[:])

            def dense(layer, b, psum):
                fo = FO_L[layer]
                aggT = aggp.tile([32, 128], bf16, tag="aggT")
                nc.scalar.copy(aggT[:], psum[:])
                zp = psZ.tile([128, fo], f32, tag="z")
                nc.tensor.matmul(
                    zp[:], lhsT=aggT[:], rhs=w_t[layer][:], start=True, stop=False
                )
                nc.tensor.matmul(
                    zp[:], lhsT=ones_t[:], rhs=b_t[layer][:], start=False, stop=True
                )
                r0, r1 = b * 128, (b + 1) * 128
                if layer < 2:
                    zm = zzp.tile([128, fo], f32, tag="zm")
                    nc.vector.tensor_scalar_mul(zm[:], zp[:], 0.1)
                    zz = zzp.tile([128, fo], bf16, tag="zz16")
                    nc.vector.tensor_tensor(
                        out=zz[:], in0=zp[:], in1=zm[:], op=mybir.AluOpType.max
                    )
                    nc.sync.dma_start(cc_in[layer][r0:r1, :], zz[:])
                else:
                    zz = zzp.tile([128, fo], f32, tag="zz32")
                    nc.scalar.copy(zz[:], zp[:])
                    nc.sync.dma_start(out[r0:r1, :], zz[:])

            for layer in range(3):
                fo = FO_L[layer]
                for p0 in range(0, NBLK, GRP):
                    blocks = list(range(p0, min(p0 + GRP, NBLK)))
                    a = int(cs[blocks[0]])  # first chunk of group
                    k = int(cs[blocks[-1] + 1] - cs[blocks[0]])  # chunks in group
                    g = gatp.tile([128, KGMAX * 128], bf16, tag="g")
                    g3 = g[:, : k * 128].rearrange("p (c f) -> p c f", f=128)
                    nc.gpsimd.dma_gather(
                        out_ap=g3,
                        in_ap=src_ap(layer),
                        idxs_ap=gidx_t[:, a * 8 : (a + k) * 8],
                        num_idxs=k * 128,
                        num_idxs_reg=k * 128,
                        elem_size=128,
                        single_packet=False,
                        queue_num=(p0 // GRP) % NQ,
                    )
                    for b in blocks:
                        kbb = int(kb[b])
                        ab = int(cs[b]) - a  # chunk offset within group
                        # per-block dst one-hot (finer pipelining than per-group)
                        sb = sp_.tile([128, KBMAX * 128], bf16, tag="s")
                        s3 = sb[:, : kbb * 128].rearrange("p (c n) -> p c n", n=128)
                        idsl = ids_t[:, cs[b] : cs[b] + kbb]
                        in1 = bass.AP(
                            idsl.tensor, idsl.offset, [idsl.ap[0], idsl.ap[1], [0, 128]]
                        )
                        in0 = iota_t[:, : kbb * 128].rearrange("p (c n) -> p c n", n=128)
                        nc.vector.tensor_tensor(
                            out=s3, in0=in0, in1=in1, op=mybir.AluOpType.is_equal
                        )
                        psum = psA.tile([32, 128], f32, tag="agg")
                        for ci in range(kbb):
                            cc = ab + ci
                            j = int(ch_band[a + cc])
                            nc.tensor.matmul(
                                psum[:],
                                lhsT=g3[:, cc, 32 * j : 32 * j + 32],
                                rhs=s3[:, ci, :],
                                start=(ci == 0),
                                stop=(ci == kbb - 1),
                            )
                        dense(layer, b, psum)

                if layer < 2:
                    nc.gpsimd.collective_compute(
                        "AllGather",
                        mybir.AluOpType.bypass,
                        ins=[cc_in[layer][:]],
                        outs=[cc_out[layer][:]],
                        replica_groups=[list(range(NC))],
                    )
                    # gathers from the Shared window are ~60% slower than
                    # from plain local DRAM; stage a local copy through SBUF
                    # (DRAM->DRAM DMA corrupts; SBUF staging is tracked+safe).
                    CW = (NC * SP * F) // 128  # 25088 cols as [128, CW] bf16
                    for r in range(8):
                        stg = stgp.tile([128, CW // 8], bf16, tag="stg")
                        src = bass.AP(
                            cc_out[layer][:].tensor, r * (CW // 8),
                            [[CW, 128], [1, CW // 8]],
                        )
                        dst = bass.AP(
                            cc_loc[layer][:].tensor, r * (CW // 8),
                            [[CW, 128], [1, CW // 8]],
                        )
                        nc.sync.dma_start(stg[:], src)
                        nc.sync.dma_start(dst, stg[:])
    nc.compile()
    return nc


# ------------------------------------------------------------------- driver
def kernel(**inputs):
    _install_birpatch()
    import ml_dtypes

    bf = ml_dtypes.bfloat16
    x = np.asarray(inputs["x"], np.float32)
    src = np.asarray(inputs["src"], np.int64)
    dst = np.asarray(inputs["dst"], np.int64)
    Ws = [np.asarray(inputs[k], np.float32) for k in ("W1", "W2", "W3")]
    bs = [np.asarray(inputs[k], np.float32) for k in ("b1", "b2", "b3")]

    key = hash((src.tobytes(), dst.tobytes()))
    if key not in _cache:
        plan, per_core = _build_plan(src, dst)
        nc = _build_nc(plan)
        _cache[key] = (nc, plan, per_core)
    nc, plan, per_core = _cache[key]

    xz = np.zeros((NC * SP, F), np.float32)
    for c in range(NC):
        xz[c * SP : c * SP + OWN] = x[c * OWN : (c + 1) * OWN]
    xzv = xz.astype(bf).reshape(ZQ, 128)
    iota = np.tile(np.arange(128), (128, KGMAX)).astype(bf)
    ones = np.ones((1, 128), bf)

    in_maps = []
    for c in range(NC):
        pc = per_core[c]
        m = {
            "xz": xzv,
            "gz": pc["gz"],
            "ids": pc["ids"].astype(bf),
            "iota": iota,
            "ones": ones,
        }
        for i in range(3):
            m[f"w{i}"] = Ws[i].astype(bf)
            m[f"b{i}"] = bs[i].astype(bf)[None, :]
        in_maps.append(m)

    from concourse.bass_utils import run_bass_kernel_spmd

    trace = os.environ.get("GCN_TRACE") == "1"
    res = run_bass_kernel_spmd(nc, in_maps, core_ids=list(range(NC)), trace=trace)
    global last_exec_ns
    last_exec_ns = res.exec_time_ns

    out = np.empty((N, FO_L[2]), np.float32)
    for c in range(NC):
        out[c * OWN : (c + 1) * OWN] = res.results[c]["out"][:OWN]
    return out
